# revision 1
# baseline (speedup 1.0000x reference)
"""Adaptive-input-embedding Bass kernel for one TRN2 chip (8 NeuronCores).

Strategy: token-parallel across the 8 cores — the 32768 tokens are grouped by
bucket, sorted by table index, and dealt as contiguous runs to the cores, so
every core processes ~4096 tokens with identical compile-time structure.
Tables and projection matrices are replicated, except that each core receives
only a <=32k-row *window* of the 237k-row tail-bucket table covering its run
(this keeps gather indices within int16 for the DMA-gather engine).

Device side: per bucket, dma_gather(transpose=True) calls (<=768 indices
each) pull the bf16 embedding rows into SBUF already transposed ([d, tokens]
chunks, i.e. matmul lhsT layout); per 128-token tile the d/128 chunk matmuls
accumulate into PSUM fp32 against the resident bf16 projection chunks; PSUM
is copied to SBUF (alternating DVE/ACT) and written out with large
contiguous partition-major DMA stores (alternating the two HWDGE rings).
The host scatters the returned rows to their token positions while
assembling the full output (the unshard step).
"""

import sys

import numpy as np

try:
    import concourse  # noqa: F401
except ImportError:
    sys.path.insert(0, "/opt/trn_rl_repo")

import ml_dtypes
from concourse import bacc, mybir, tile
from concourse.bass_utils import run_bass_kernel_spmd

BUCKETS = (0, 300, 3000, 30000, 267734)
SIZES = [BUCKETS[i + 1] - BUCKETS[i] for i in range(4)]
D = 1024
DS = [1024, 512, 256, 128]  # embedding dim per bucket
KS = [8, 4, 2, 1]  # 128-chunks per bucket
WOFF = [0, 8, 12, 14]  # chunk offset of each bucket in wcat
NCHUNK = 15
SUB = 32768  # rows addressable by one int16 gather call
NCORES = 8
SEQ = 4096
NTOK = NCORES * SEQ
P = 128
GB = 8  # tiles per store batch

MODE = "seq_bf16"

_BF16 = ml_dtypes.bfloat16

_cache: dict = {}


def _r16(v):
    return -(-int(v) // 16) * 16


def _r128(v):
    return -(-int(v) // 128) * 128


class Plan:
    pass


def _plan(x):
    """Global bucketing + even dealing of each bucket across the cores.

    Bucket 3 (237k rows) is dealt as contiguous runs of the index-sorted
    token list, so each core's gather indices span < 32k table rows and fit
    int16 against a per-core window of the table (passed as that core's e3
    input). Produces identical compile-time structure for all cores."""
    xf = x.reshape(-1).astype(np.int64)
    assert xf.shape[0] == NTOK
    bkt = np.searchsorted(np.asarray(BUCKETS), xf, side="right") - 1
    bkt = np.clip(bkt, 0, 3)
    loc = xf - np.asarray(BUCKETS)[bkt]

    # per-(bucket, core) token positions: sort by table index, deal
    # contiguous runs (counts differ by <=1, spans stay narrow for bucket 3)
    per_core_pos = {}
    wbase = np.zeros((4, NCORES), np.int64)  # per-core table window base
    alloc = [0] * 4
    wrows = [0] * 4  # table window rows (compile-time shape)
    for b in range(4):
        pos = np.nonzero(bkt == b)[0]
        pos = pos[np.argsort(loc[pos], kind="stable")]
        n = pos.size
        cnt = np.full(NCORES, n // NCORES)
        cnt[: n % NCORES] += 1
        cuts = np.concatenate([[0], np.cumsum(cnt)])

        def spans(cuts_):
            sp, mx = 0, 0
            for c in range(NCORES):
                pc = pos[cuts_[c] : cuts_[c + 1]]
                if pc.size:
                    sp = max(sp, int(loc[pc[-1]] - loc[pc[0]]) + 1)
                    mx = max(mx, pc.size)
            return sp, mx

        span, mxc = spans(cuts)
        if b == 3 and span > SUB:
            # skewed distribution: balanced cuts straddle >32k-row ranges;
            # fall back to fixed 32k-row boundary cuts (unbalanced counts
            # but indices stay int16 against each core's window)
            edges = np.searchsorted(loc[pos], np.arange(1, NCORES) * SUB)
            cuts = np.concatenate([[0], edges, [n]])
            span, mxc = spans(cuts)
        for c in range(NCORES):
            pc = pos[cuts[c] : cuts[c + 1]]
            per_core_pos[(b, c)] = pc
            if pc.size:
                wbase[b, c] = loc[pc[0]]
        alloc[b] = int(_r16(mxc))
        wrows[b] = min(span if b == 3 else SIZES[b], SIZES[b])
        wrows[b] = max(wrows[b], 1)
        assert wrows[b] <= SUB, (b, wrows[b])
        if b < 3:
            wbase[b] = 0

    # slot layout: one 128-aligned block per bucket
    segs = []  # (bucket, o_slot, n_alloc, num_idxs)
    blocks = []
    o = 0
    for b in range(4):
        ni = _r128(alloc[b])
        segs.append((b, o, alloc[b], ni))
        blocks.append((o, ni))
        o += ni
    ntot = o

    p = Plan()
    p.segs, p.blocks, p.ntot = segs, blocks, ntot
    p.t_total = ntot // P
    p.alloc = alloc
    p.wrows = wrows
    p.wbase = wbase

    gidx = np.zeros((NCORES, P, ntot // 16), np.int16)
    rowpos = np.full((NCORES, ntot), -1, np.int64)  # slot -> global token pos
    for b, o, na, ni in segs:
        for c in range(NCORES):
            pos = per_core_pos[(b, c)]
            n = pos.size
            li = np.zeros(na, np.int64)
            li[:n] = loc[pos] - wbase[b, c]
            rowpos[c, o : o + n] = pos
            ii = np.arange(na)
            cols = o // 16 + ii // 16
            rows = ii % 16
            for g in range(8):  # replicate across the 8 groups of 16 partitions
                gidx[c, g * 16 + rows, cols] = li.astype(np.int16)
    p.gidx, p.rowpos = gidx, rowpos
    return p


def _build(plan, mode=MODE, repeat=1, loop_n=None, b3_first=True, split_w=True, gbatch=16, zbufs=3, psbufs=4, store_split=True, tail_dve=True):
    """Build + compile the SPMD Bass program.

    repeat>1 re-emits the whole body; loop_n wraps the body in a HW For_i
    loop (both used only for differential timing)."""
    ntot, t_total = plan.ntot, plan.t_total
    bf16 = mybir.dt.bfloat16
    f32 = mybir.dt.float32
    odt = bf16 if mode.endswith("bf16") else f32

    nc = bacc.Bacc(None, target_bir_lowering=False)
    e_d = [
        nc.declare_dram_parameter(f"e{b}", [plan.wrows[b], DS[b]], bf16, isOutput=False)
        for b in range(4)
    ]
    wcat_d = nc.declare_dram_parameter("wcat", [P, NCHUNK * D], bf16, isOutput=False)
    gidx_d = nc.declare_dram_parameter("gidx", [P, ntot // 16], mybir.dt.int16, isOutput=False)
    # partition-major: slot s lives at out[s % 128, s // 128, :] so each
    # partition's store stream is contiguous (few, large descriptors)
    out_d = nc.declare_dram_parameter("out", [P, t_total, D], odt, isOutput=True)

    bbase = [blk[0] for blk in plan.blocks]
    bslots = [blk[1] for blk in plan.blocks]

    with tile.TileContext(nc) as tc:
        with (
            tc.tile_pool(name="const", bufs=1) as cp,
            tc.tile_pool(name="gbuf", bufs=1) as gp,
            tc.tile_pool(name="zbuf", bufs=zbufs) as zp,
            tc.tile_pool(name="ps", bufs=psbufs, space="PSUM") as pp,
        ):
            gidx = cp.tile([P, ntot // 16], mybir.dt.int16)
            nc.gpsimd.dma_start(out=gidx[:], in_=gidx_d[:])
            wcat = cp.tile([P, NCHUNK * D], bf16)
            if split_w:
                # W3 first: bucket-3 tiles are processed first and its W slice
                # is small, so the first matmuls aren't gated on the full load
                upfront = (3,) if split_w == 3 else (3, 2, 1, 0)
                for b in upfront:
                    sl = slice(WOFF[b] * D, (WOFF[b] + KS[b]) * D)
                    eng = nc.sync if (split_w is True or split_w in (1, 3) or b >= 2) else nc.scalar
                    eng.dma_start(out=wcat[:, sl], in_=wcat_d[:, sl])
            else:
                nc.sync.dma_start(out=wcat[:], in_=wcat_d[:])

            gt = [
                gp.tile([P, KS[b], bslots[b]], bf16, name=f"G{b}", tag=f"G{b}")
                if bslots[b]
                else None
                for b in range(4)
            ]

            def bucket_of_tile(t):
                slot = t * P
                for b in range(4):
                    if bbase[b] <= slot < bbase[b] + bslots[b]:
                        return b
                raise AssertionError(t)

            def body(_iv=None, unroll=1):
                deferred_w = []
                if split_w == 3:
                    for b in (2, 1, 0):
                        sl = slice(WOFF[b] * D, (WOFF[b] + KS[b]) * D)
                        deferred_w.append(sl)
                GCAP = 768  # >=1024 idxs in one SWDGE gather wedges the device
                for b, o, na, ni in (
                    sorted(plan.segs, key=lambda e: e[0] != 3)
                    if b3_first
                    else plan.segs
                ):
                    for k in range(0, ni, GCAP):
                        nk = min(GCAP, ni - k)
                        ok = o + k
                        o_local = ok - bbase[b]
                        nc.gpsimd.dma_gather(
                            out_ap=gt[b][:, :, o_local : o_local + nk],
                            in_ap=e_d[b][:],
                            idxs_ap=gidx[:, ok // 16 : ok // 16 + nk // 16],
                            num_idxs=nk,
                            num_idxs_reg=nk,
                            elem_size=DS[b],
                            transpose=True,
                        )

                # bucket-3 tiles first (largest block, cheapest W), then 0..2
                t3 = bbase[3] // P
                if b3_first:
                    order = list(range(t3, t_total)) + list(range(0, t3))
                else:
                    order = list(range(t_total))
                oi = 0
                while oi < len(order):
                    t = order[oi]
                    gb = 1
                    for g in range(1, min(gbatch, len(order) - oi)):
                        if order[oi + g] == t + g:
                            gb += 1
                        else:
                            break
                    zt = zp.tile([P, gb, D], odt, tag="z")
                    for g in range(gb):
                        tt = t + g
                        b = bucket_of_tile(tt)
                        ts0 = tt * P - bbase[b]
                        ps = pp.tile([P, D], f32, tag="ps")
                        kb = KS[b]
                        for c in range(kb):
                            lhsT = gt[b][:, c, ts0 : ts0 + P]
                            for h in range(2):
                                nc.tensor.matmul(
                                    out=ps[:, h * 512 : (h + 1) * 512],
                                    lhsT=lhsT,
                                    rhs=wcat[:, (WOFF[b] + c) * D + h * 512 :][:, :512],
                                    start=(c == 0),
                                    stop=(c == kb - 1),
                                )
                        last_batch = oi + gb >= len(order)
                        if tail_dve == 2 and not last_batch:
                            nc.vector.tensor_copy(out=zt[:, g, :512], in_=ps[:, :512])
                            nc.scalar.copy(out=zt[:, g, 512:], in_=ps[:, 512:])
                        elif tt % 2 == 0 or (tail_dve and last_batch):
                            nc.vector.tensor_copy(out=zt[:, g, :], in_=ps[:])
                        else:
                            nc.scalar.copy(out=zt[:, g, :], in_=ps[:])
                    last_batch_s = oi + gb >= len(order)
                    if store_split == 2:
                        seng = nc.scalar if last_batch_s else nc.sync
                    else:
                        seng = nc.scalar if (store_split and (t // gbatch) % 2) else nc.sync
                    seng.dma_start(out=out_d[:, t : t + gb, :], in_=zt[:])
                    for sl in deferred_w:
                        nc.sync.dma_start(out=wcat[:, sl], in_=wcat_d[:, sl])
                    deferred_w = []
                    oi += gb

            if loop_n is None:
                for _ in range(repeat):
                    body()
            else:
                with tc.For_i(0, loop_n, 1) as _i:
                    body()
    nc.compile()
    return nc


def _prep_inputs(embs, ws, plan, mode=MODE):
    wcat = np.zeros((P, NCHUNK * D), _BF16)
    for b in range(4):
        for c in range(KS[b]):
            wcat[:, (WOFF[b] + c) * D : (WOFF[b] + c + 1) * D] = ws[b][
                c * P : (c + 1) * P, :
            ].astype(_BF16)
    ebf = [e.astype(_BF16) for e in embs]
    in_maps = []
    for c in range(NCORES):
        m = {}
        for b in range(4):
            base = int(plan.wbase[b, c])
            w = plan.wrows[b]
            win = ebf[b][base : base + w]
            if win.shape[0] < w:  # window runs past the table end: zero-pad
                win = np.concatenate(
                    [win, np.zeros((w - win.shape[0], DS[b]), _BF16)]
                )
            m[f"e{b}"] = np.ascontiguousarray(win)
        m["wcat"] = wcat
        m["gidx"] = np.ascontiguousarray(plan.gidx[c])
        in_maps.append(m)
    return in_maps


def _assemble(plan, mode, results, repeat=1):
    out = np.empty((NTOK, D), np.float32)
    for c in range(NCORES):
        r = results[c]["out"]  # [128, T, D] partition-major
        r = np.ascontiguousarray(r.transpose(1, 0, 2)).reshape(-1, D)
        valid = plan.rowpos[c] >= 0
        out[plan.rowpos[c][valid]] = r[valid].astype(np.float32)
    return out.reshape(NCORES, SEQ, D)


def run(inputs, mode=MODE, trace=False):
    x = np.asarray(inputs["x"])
    embs = [np.asarray(inputs[f"emb{b}"]) for b in range(4)]
    ws = [np.asarray(inputs[f"W{b}"]) for b in range(4)]
    assert x.shape == (NCORES, SEQ), x.shape

    plan = _plan(x)
    key = (tuple(plan.alloc), tuple(plan.wrows), mode)
    if key not in _cache:
        _cache[key] = _build(plan, mode)
    nc = _cache[key]

    in_maps = _prep_inputs(embs, ws, plan, mode)
    res = run_bass_kernel_spmd(
        nc, in_maps, core_ids=list(range(NCORES)), trace=trace
    )
    out = _assemble(plan, mode, res.results)
    return out, res


def kernel(**inputs):
    out, _ = run(inputs, mode=MODE, trace=False)
    return out

